# revision 25
# baseline (speedup 1.0000x reference)
"""Trainium2 Bass kernel for nn_BoundaryLoss (BCE over 3x3 boundary maps).

Self-contained: hardcodes shapes [8,2,1024,1024] pred f32 / [8,1024,1024]
target int64-or-int32. Shards batch across 8 NeuronCores (1 image/core).

Math: with 2 classes both class-loops of the reference produce the SAME
boundary map b (values {0,1}); after the remove-long-lines kill the map is
2*b or 0. BCE-with-logits mean then reduces to a closed form over four
per-image statistics:
    kill_p / kill_t : any column-sum(over H) of b >= 300
    cp              : count of b_p
    cpt             : count of b_p AND b_t
Per image (n = H*W):
    kill_p           -> S = n*ln2
    else             -> S = (n-cp)*ln2 + cp*(2+log1p(e^-2)) - 4*(kill_t ? 0 : cpt)
loss = sum(S) / (B*n)

Device pipeline per image (row-slab layout, partitions = H rows):
  masks: m_p = (pred1 > pred0), m_t = low word of target  (bf16 {0,1})
  hsum: horizontal replicate-pad 3-window sum (DVE shifted adds)
  C = band @ hsum on PE (vertical replicate-pad weights in band lhsT)
  b = ((C-4.5)^2 <= 12.5)  via ACT Square + DVE is_le
  colsums over H via PE ones-matmul accumulated in PSUM
  cpt row-sums via DVE tensor_tensor_reduce
Boundary condition: replicate padding is max/min/count-equivalent to the
reference's in-bounds pooling (weighted window-sum in [0,9], ==0 iff all-0,
==9 iff all-1).
"""
import math
from contextlib import ExitStack

import numpy as np
import ml_dtypes

import concourse.bass as bass
import concourse.bacc as bacc
import concourse.mybir as mybir
import concourse.tile as tile
from concourse.bass_utils import run_bass_kernel_spmd

BF16 = mybir.dt.bfloat16
F32 = mybir.dt.float32
I32 = mybir.dt.int32

B, H, W = 8, 1024, 1024
NPIX = H * W
THR = 300.0
LN2 = math.log(2.0)
C2 = math.log1p(math.exp(-2.0))

# slabs: 8 x 126 + 1 x 16 output rows; input rows include +-1 halo
SLABS = []
s = 0
while s * 126 < H:
    o0 = s * 126
    o1 = min(H, o0 + 126)
    i0 = max(0, o0 - 1)
    i1 = min(H, o1 + 1)
    SLABS.append((o0, o1, i0, i1))
    s += 1
NSLAB = len(SLABS)  # 9

# engine choices (tunable): "gpsimd" offloads DVE; fall back to "vector" if HW disagrees
CFG = {
    # engine for m_p = pred1 > pred0 (Pool has no TensorTensor in the ISA)
    "isgt": "vector",
    # engine for the int64-path strided cast int32->bf16
    "cast": "vector",
    # engine for b = (s <= 12.5)
    "cmp": "vector",
    # int32-target path: cast during DMA (SWDGE) instead of an engine op
    "int32_dma_cast": False,
    # debug bisection: "full" | "nosq" (Square->Copy) | "basic" (nosq + no
    # colsum matmuls + no TTR) | "nottr" (no TTR) | "nocs" (no colsum MMs)
    "stage": "full",
}


def _build_band_lhsT():
    """bandT [128, NSLAB*128] bf16: per-slab vertical replicate-pad weights."""
    bandT = np.zeros((128, NSLAB * 128), np.float32)
    for si, (o0, o1, i0, i1) in enumerate(SLABS):
        for m in range(o1 - o0):
            r = o0 + m
            for dr in (-1, 0, 1):
                rr = min(H - 1, max(0, r + dr))
                bandT[rr - i0, 128 * si + m] += 1.0
    return bandT.astype(ml_dtypes.bfloat16)


def _build_nc(tgt_cols, tgt_step):
    """Build the per-core Bass program.

    tgt_cols/tgt_step: 2048/2 when target arrives as int64 (viewed as int32
    pairs; low word at even columns), 1024/1 when it arrives as int32.
    """
    nc = bacc.Bacc("TRN2", target_bir_lowering=False, debug=False)

    def register_const(value, dtype=F32):
        t = nc.alloc_sbuf_tensor(f"const-{dtype.name}-{value}", [128, 1], dtype)
        nc.gpsimd.memset(t.ap(), value)
        nc.const_aps.aps[(dtype, value)] = t.ap()

    register_const(-4.5)
    nc.all_engine_barrier()

    pred = nc.dram_tensor("pred", [2, H, W], F32, kind="ExternalInput").ap()
    pred_hcw = pred.rearrange("c h w -> h c w")
    tgt = nc.dram_tensor("tgt", [H, tgt_cols], I32, kind="ExternalInput").ap()
    bandT_d = nc.dram_tensor("bandT", [128, NSLAB * 128], BF16,
                             kind="ExternalInput").ap()
    ones_d = nc.dram_tensor("ones", [128, 1], BF16, kind="ExternalInput").ap()
    colsums_o = nc.dram_tensor("colsums", [3, W], F32, kind="ExternalOutput").ap()

    AT = mybir.AluOpType
    AF = mybir.ActivationFunctionType

    eng = {"gpsimd": nc.gpsimd, "vector": nc.vector}

    with tile.TileContext(nc) as tc, ExitStack() as ctx:
        const_pool = ctx.enter_context(tc.tile_pool(name="const", bufs=1))
        pred_pool = ctx.enter_context(tc.tile_pool(name="pred", bufs=4))
        tgt_pool = ctx.enter_context(tc.tile_pool(name="tgt", bufs=2))
        mask_pool = ctx.enter_context(tc.tile_pool(name="mask", bufs=4))
        hs_pool = ctx.enter_context(tc.tile_pool(name="hs", bufs=4))
        scr_pool = ctx.enter_context(tc.tile_pool(name="scr", bufs=3))
        sq_pool = ctx.enter_context(tc.tile_pool(name="sq", bufs=4))
        b_pool = ctx.enter_context(tc.tile_pool(name="b", bufs=5))
        prod_pool = ctx.enter_context(tc.tile_pool(name="prod", bufs=4))
        out_pool = ctx.enter_context(tc.tile_pool(name="out", bufs=1))
        psum_c = ctx.enter_context(tc.tile_pool(name="psc", bufs=6, space="PSUM"))
        psum_cs = ctx.enter_context(tc.tile_pool(name="pscs", bufs=1, space="PSUM"))

        bandT = const_pool.tile([128, NSLAB * 128], BF16)
        nc.sync.dma_start(bandT[:], bandT_d)
        ones = const_pool.tile([128, 1], BF16)
        nc.sync.dma_start(ones[:], ones_d)

        colsums = psum_cs.tile([65, W], F32)
        deferred = []

        # pair uniform 128-row slabs (1,2),(3,4),(5,6) into single DMAs:
        # custom AP with an overlapping 126-row slab stride halves the
        # transfer count (fixed per-DMA latency is partially exposed)
        PAIR_FIRST = {1: 2, 3: 4, 5: 6}
        paired = {}

        def paired_load(si, i0):
            t2 = tgt_pool.tile([128, 2 * tgt_cols], I32, tag="t32")
            in_t = tgt[i0:i0 + 128, :].copy()
            dt_ = list(in_t.ap)
            in_t.ap = mybir.VecI64Pair(
                [list(dt_[0]), [126 * tgt_cols, 2], list(dt_[1])])
            nc.sync.dma_start(
                t2[0:128, :].rearrange("p (s w) -> p s w", s=2), in_t)
            return t2

        prev_deferred = []
        for si, (o0, o1, i0, i1) in enumerate(SLABS):
            n_in = i1 - i0
            n_out = o1 - o0
            start = si == 0
            stop = si == NSLAB - 1

            p01 = pred_pool.tile([128, 2 * W], F32, tag="p01")
            nc.sync.dma_start(p01[0:n_in, :], pred_hcw[i0:i1, :, :])
            if si in PAIR_FIRST:
                t2 = paired_load(si, i0)
                paired[si] = t2[:, 0:tgt_cols]
                paired[si + 1] = t2[:, tgt_cols:2 * tgt_cols]
            if si in paired:
                t32 = paired.pop(si)
            else:
                t32 = tgt_pool.tile([128, tgt_cols], I32, tag="t32s")
                nc.sync.dma_start(t32[0:n_in, :], tgt[i0:i1, :])

            # masks live at cols 1..1024 of a [128, W+2] tile; cols 0 and
            # W+1 replicate the edges so the hsum add needs no fixups and
            # reads 4B-aligned operands (2x DVE mode)
            m_p = mask_pool.tile([128, W + 2], BF16, tag="mp")
            eng[CFG["isgt"]].tensor_tensor(
                m_p[0:n_in, 1:W + 1], p01[0:n_in, W:2 * W], p01[0:n_in, 0:W],
                AT.is_gt)
            m_t = mask_pool.tile([128, W + 2], BF16, tag="mt")
            if t32 is None:
                nc.gpsimd.dma_start(m_t[0:n_in, 1:W + 1], tgt[i0:i1, :])
            elif CFG["cast"] == "scalar":
                nc.scalar.copy(m_t[0:n_in, 1:W + 1],
                               t32[0:n_in, 0:tgt_cols:tgt_step])
            else:
                eng[CFG["cast"]].tensor_copy(
                    m_t[0:n_in, 1:W + 1], t32[0:n_in, 0:tgt_cols:tgt_step])
            for m in (m_p, m_t):
                nc.vector.tensor_copy(m[0:n_in, 0:1], m[0:n_in, 1:2])
                nc.vector.tensor_copy(m[0:n_in, W + 1:W + 2], m[0:n_in, W:W + 1])

            # per-map S = m[j-1]+m[j+1] (aligned 2x DVE)
            Ss = []
            for mi, m in enumerate((m_p, m_t)):
                S = scr_pool.tile([128, W], BF16, tag=f"S{mi}")
                nc.vector.tensor_add(
                    S[0:n_in, :], m[0:n_in, 0:W], m[0:n_in, 2:W + 2])
                Ss.append(S)

            # back-end in independent 512-wide halves for finer overlap:
            # C = band@S + band@m (1-bank PSUM), square, cmp, colsum MMs
            lhs = bandT[0:n_in, 128 * si:128 * si + n_out]
            for h0 in (0, 512):
                bhalf = []
                for mi in (0, 1):
                    m, S = (m_p, m_t)[mi], Ss[mi]
                    C = psum_c.tile([126, 512], F32, tag="C")
                    nc.tensor.matmul(
                        C[0:n_out, :], lhs, S[0:n_in, h0:h0 + 512],
                        start=True, stop=False, skip_group_check=True)
                    nc.tensor.matmul(
                        C[0:n_out, :], lhs, m[0:n_in, 1 + h0:1 + h0 + 512],
                        start=False, stop=True, skip_group_check=True)
                    sq = sq_pool.tile([126, 512], BF16, tag=f"s{mi}")
                    if CFG["stage"] == "full":
                        nc.scalar.activation(sq[0:n_out, :], C[0:n_out, :],
                                             AF.Square, bias=-4.5, scale=1.0)
                    else:
                        nc.scalar.copy(sq[0:n_out, :], C[0:n_out, :])
                    bt = b_pool.tile([126, 512], BF16, tag=f"b{mi}")
                    eng[CFG["cmp"]].tensor_single_scalar(
                        bt[0:n_out, :], sq[0:n_out, :], 12.5, AT.is_le)
                    bhalf.append(bt)
                    if CFG["stage"] not in ("basic", "nocs"):
                        deferred.append(
                            (lambda mi=mi, h0=h0, bt=bt, n_out=n_out,
                                    start=start, stop=stop:
                             nc.tensor.matmul(
                                 colsums[32 * mi:32 * mi + 1, h0:h0 + 512],
                                 ones[0:n_out, :], bt[0:n_out, :],
                                 start=start, stop=stop,
                                 skip_group_check=True)))
                if CFG["stage"] in ("basic", "nottr"):
                    continue
                prod = prod_pool.tile([126, 512], BF16, tag="prod")
                nc.vector.tensor_mul(
                    prod[0:n_out, :], bhalf[0][0:n_out, :], bhalf[1][0:n_out, :])
                deferred.append(
                    (lambda h0=h0, prod=prod, n_out=n_out, start=start,
                            stop=stop:
                     nc.tensor.matmul(
                         colsums[64:65, h0:h0 + 512],
                         ones[0:n_out, :], prod[0:n_out, :],
                         start=start, stop=stop, skip_group_check=True)))

            for th in prev_deferred:
                th()
            prev_deferred = deferred
            deferred = []

        for th in prev_deferred:
            th()

        cs_sb = out_pool.tile([65, W], F32)
        if CFG["stage"] in ("basic", "nocs"):
            for r in (0, 32, 64):
                nc.vector.memzero(cs_sb[r:r + 1, :])
        else:
            for r in (0, 32, 64):
                nc.scalar.copy(cs_sb[r:r + 1, :], colsums[r:r + 1, :])
        for oi, r in enumerate((0, 32, 64)):
            nc.sync.dma_start(colsums_o[oi:oi + 1, :], cs_sb[r:r + 1, :])

    nc.compile()
    return nc


_NC_CACHE = {}


def _get_nc(tgt_cols, tgt_step):
    key = (tgt_cols, tgt_step, tuple(sorted(CFG.items())))
    if key not in _NC_CACHE:
        _NC_CACHE[key] = _build_nc(tgt_cols, tgt_step)
    return _NC_CACHE[key]


def _prep_inputs(pred, target):
    pred = np.asarray(pred)
    if pred.dtype != np.float32:
        pred = pred.astype(np.float32)
    pred = np.ascontiguousarray(pred)
    assert pred.shape == (B, 2, H, W), pred.shape

    target = np.asarray(target)
    assert target.shape == (B, H, W), target.shape
    if target.dtype == np.int64:
        t32 = np.ascontiguousarray(target).view(np.int32).reshape(B, H, 2 * W)
        tgt_cols, tgt_step = 2 * W, 2
    elif target.dtype == np.int32:
        t32 = np.ascontiguousarray(target)
        tgt_cols, tgt_step = W, 1
    else:
        t32 = np.ascontiguousarray(target.astype(np.int32))
        tgt_cols, tgt_step = W, 1
    return pred, t32, tgt_cols, tgt_step


def _run(pred, target, trace=False, trace_kwargs=None):
    pred, t32, tgt_cols, tgt_step = _prep_inputs(pred, target)
    nc = _get_nc(tgt_cols, tgt_step)

    bandT_np = _build_band_lhsT()
    ones_np = np.ones((128, 1), ml_dtypes.bfloat16)
    in_maps = [
        {"pred": pred[i], "tgt": t32[i], "bandT": bandT_np, "ones": ones_np}
        for i in range(B)
    ]
    res = run_bass_kernel_spmd(nc, in_maps, list(range(B)), trace=trace,
                               **(trace_kwargs or {}))

    total = 0.0
    for i in range(B):
        cs = np.asarray(res.results[i]["colsums"], np.float64)
        colsum_p, colsum_t, colsum_pt = cs[0], cs[1], cs[2]
        kill_p = colsum_p.max() >= THR
        kill_t = colsum_t.max() >= THR
        if kill_p:
            total += NPIX * LN2
        else:
            cp = colsum_p.sum()
            cpt = 0.0 if kill_t else colsum_pt.sum()
            total += (NPIX - cp) * LN2 + cp * (2.0 + C2) - 4.0 * cpt
    loss = np.float32(total / (B * NPIX))
    return loss, res


def kernel(pred, target):
    return _run(pred, target)[0]
